# revision 1
# baseline (speedup 1.0000x reference)
"""Trainium2 Bass kernel for nn_Backward_14328010900205.

Flow-warp (grid_sample bilinear, zero padding, align_corners=True) with a
hard validity mask, matching the jax reference:

    (y, x) = (i + fy[b,i,j], j + fx[b,i,j])
    out[b,c,i,j] = mask(b,i,j) * sum_k w_k(b,i,j) * img[b,c, y_k, x_k]

Sharding: pure data parallel, one batch sample per NeuronCore (B=8 -> 8 cores).

Per-core device algorithm:
  - flow -> per-pixel quad weights W4 (f32, natural [H,W] layout) and flat
    gather-index maps (uint16, GPSIMD 16-partition-wrapped layout, which
    unwraps to natural raster order).
  - image processed in 2 channel chunks (128 + 68 ch) x 4 row-window passes
    (<=48 input rows resident in SBUF at a time).
  - GPSIMD indirect_copy fetches the 4 bilinear corners per output pixel as
    two (x, x+1) pairs into a quad-interleaved tile.
  - PE broadcasts per-row quad weights across 128 partitions (ones-matmul
    into PSUM); DVE multiplies in place and reduces 4->1 with an add tree;
    DMA stores output rows.
"""

import numpy as np

from concourse import bass, mybir, tile

AOT = mybir.AluOpType
F32 = mybir.dt.float32
U16 = mybir.dt.uint16

B, C, H, W = 8, 196, 128, 256
HW = H * W
N_CORES = 8
MARGIN = 7  # max |floor(flow_y)| + 1 supported by the row-window passes

# (out_lo, out_hi, win_lo, win_hi): out spans multiples of CALL_ROWS, window
# <= WIN_MAX rows and covers [out_lo - MARGIN, out_hi + MARGIN) clip [0, H).
WIN_MAX = 48
# (out_lo, out_hi, win_lo): window is always WIN_MAX rows from win_lo
PASSES = [
    (0, 32, 0),
    (32, 64, 25),
    (64, 96, 57),
    (96, 128, 80),
]
# (first channel, n channels): chunk 2 loads only 68 channels; tile
# partitions [68:128) are memset once per pass (gathered but not stored)
CHUNKS = [(0, 128), (128, 68)]
CALL_ROWS = 4


def _check_passes():
    cover = 0
    for (o0, o1, w0) in PASSES:
        assert o0 == cover
        cover = o1
        assert (o1 - o0) % CALL_ROWS == 0
        assert w0 <= max(0, o0 - MARGIN) and w0 + WIN_MAX >= min(H, o1 + MARGIN)
        assert 0 <= w0 and w0 + WIN_MAX <= H
    assert cover == H
    for (ch0, cskip) in CHUNKS:
        assert ch0 + 128 <= C + (128 - (C - ch0)) and ch0 + 128 >= C or ch0 == 0


_check_passes()


def split_drain_waits(nc, max_waits=1):
    """walrus CoreV3 codegen rejects instructions carrying more than a
    couple of sync waits; hoist extras onto preceding NoOps."""
    fn = nc.main_func
    n = 0
    for bb in fn.blocks:
        insts = bb.instructions
        i = 0
        while i < len(insts):
            ins = insts[i]
            if type(ins).__name__ != "InstNoOp":
                si = ins.sync_info
                ow = list(si.on_wait) if (si and si.on_wait) else []
                if len(ow) > max_waits:
                    keep = ow[-max_waits:]
                    extras = ow[:-max_waits]
                    ins.sync_info = si.__replace__(on_wait=keep)
                    for k, wt in enumerate(extras):
                        nop = mybir.InstNoOp(
                            name=f"{ins.name}-wsplit{k}",
                            engine=ins.engine,
                            ins=[],
                            outs=[],
                            sync_info=mybir.SyncInfo(on_wait=[wt], on_update=[]),
                        )
                        insts.insert(i, nop)
                        i += 1
                        n += 1
            i += 1
    return n


class Slots:
    """Column-sliced scratch slots inside one SBUF tile (manual reuse --
    avoids Tile's 4KB-per-tag padding for many small map tiles)."""

    def __init__(self, tilebuf, width):
        self.t = tilebuf
        self.w = width

    def __getitem__(self, k):
        return self.t[:, k * self.w:(k + 1) * self.w]


def build_program(n_iters=1, split_drains=True, skip=()):
    nc = bass.Bass("TRN2", target_bir_lowering=False, debug=False)

    img = nc.declare_dram_parameter("img", [C, HW], F32, isOutput=False)
    flow = nc.declare_dram_parameter("flow", [2, H, W], F32, isOutput=False)
    Jc = nc.declare_dram_parameter("cJ", [128, 256], F32, isOutput=False)
    Ic = nc.declare_dram_parameter("cI", [128, 256], F32, isOutput=False)
    JWc = nc.declare_dram_parameter("cJW", [128, 2048], F32, isOutput=False)
    IWc = nc.declare_dram_parameter("cIW", [128, 2048], F32, isOutput=False)
    ONESc = nc.declare_dram_parameter("cONES", [1, 128], F32, isOutput=False)
    out = nc.declare_dram_parameter("out", [C, HW], F32, isOutput=True)

    with tile.TileContext(nc) as tc:
        with (
            tc.tile_pool(name="consts", bufs=1) as cpool,
            tc.tile_pool(name="maps", bufs=1) as mpool,
            tc.tile_pool(name="img", bufs=1) as ipool,
            tc.tile_pool(name="gath", bufs=2) as gpool,
            tc.tile_pool(name="outp", bufs=2) as opool,
            tc.tile_pool(name="wst", bufs=2) as wpool,
            tc.tile_pool(name="psum", bufs=4, space="PSUM") as pspool,
        ):
            cs = {}
            for name, dram, shape in (
                ("J", Jc, [128, 256]),
                ("I", Ic, [128, 256]),
                ("JW", JWc, [128, 2048]),
                ("IW", IWc, [128, 2048]),
                ("ONES", ONESc, [1, 128]),
            ):
                cs[name] = cpool.tile(shape, F32, tag="c" + name, name="c" + name)
                nc.sync.dma_start(out=cs[name][:, :], in_=dram[:, :])

            for _ in range(n_iters):
                _iteration(nc, mpool, ipool, gpool, opool, wpool, pspool,
                           img, flow, out, cs, skip)

    if split_drains:
        split_drain_waits(nc)
    return nc


def _iteration(nc, mpool, ipool, gpool, opool, wpool, pspool,
               img, flow, out, cs, skip=()):
    ts = nc.vector.tensor_scalar
    tt = nc.vector.tensor_tensor

    # ---------------- map phase ----------------
    mb = Slots(mpool.tile([128, 16 * 256], F32, tag="mapbuf", name="mapbuf"), 256)
    wb = Slots(mpool.tile([128, 4 * 2048], F32, tag="wrapbuf", name="wrapbuf"), 2048)
    W4 = mpool.tile([128, 1024], F32, tag="W4", name="W4")
    iA = mpool.tile([128, 2048], U16, tag="iA", name="iA")
    iB = mpool.tile([128, 2048], U16, tag="iB", name="iB")

    X, Y, AX, AY, X0, Y0 = mb[0], mb[1], mb[2], mb[3], mb[4], mb[5]
    P0X, P1X, P0Y, P1Y, MSK = mb[6], mb[7], mb[8], mb[9], mb[10]
    S1, S2, S3, WSA, WSB = mb[11], mb[12], mb[13], mb[14], mb[15]

    nc.sync.dma_start(out=X, in_=flow[0])
    nc.sync.dma_start(out=Y, in_=flow[1])

    tt(out=X, in0=X, in1=cs["J"][:, :], op=AOT.add)          # x = j + fx
    tt(out=Y, in0=Y, in1=cs["I"][:, :], op=AOT.add)          # y = i + fy
    # floor via round-to-nearest bias trick: r = (x + 2^23) - 2^23;
    # floor = r - [r > x]; frac = x - floor
    ts(X0, X, 8388608.0, 8388608.0, AOT.add, AOT.subtract)   # round(x)
    tt(out=AX, in0=X0, in1=X, op=AOT.is_gt)
    tt(out=X0, in0=X0, in1=AX, op=AOT.subtract)              # floor(x)
    tt(out=AX, in0=X, in1=X0, op=AOT.subtract)               # frac(x)
    ts(Y0, Y, 8388608.0, 8388608.0, AOT.add, AOT.subtract)
    tt(out=AY, in0=Y0, in1=Y, op=AOT.is_gt)
    tt(out=Y0, in0=Y0, in1=AY, op=AOT.subtract)
    tt(out=AY, in0=Y, in1=Y0, op=AOT.subtract)

    # p0x = (1-ax)*[0<=x0<=255], p1x = ax*[-1<=x0<=254]
    ts(S1, X0, 0.0, 255.0, AOT.max, AOT.min)
    tt(out=S1, in0=X0, in1=S1, op=AOT.is_equal)
    ts(S2, AX, -1.0, 1.0, AOT.mult, AOT.add)
    tt(out=P0X, in0=S1, in1=S2, op=AOT.mult)
    ts(S1, X0, -1.0, 254.0, AOT.max, AOT.min)
    tt(out=S1, in0=X0, in1=S1, op=AOT.is_equal)
    tt(out=P1X, in0=AX, in1=S1, op=AOT.mult)
    # p0y, p1y
    ts(S1, Y0, 0.0, 127.0, AOT.max, AOT.min)
    tt(out=S1, in0=Y0, in1=S1, op=AOT.is_equal)
    ts(S2, AY, -1.0, 1.0, AOT.mult, AOT.add)
    tt(out=P0Y, in0=S1, in1=S2, op=AOT.mult)
    ts(S1, Y0, -1.0, 126.0, AOT.max, AOT.min)
    tt(out=S1, in0=Y0, in1=S1, op=AOT.is_equal)
    tt(out=P1Y, in0=AY, in1=S1, op=AOT.mult)

    # mask = [(p0x+p1x)*(p0y+p1y) > 0.999]
    tt(out=S1, in0=P0X, in1=P1X, op=AOT.add)
    tt(out=S2, in0=P0Y, in1=P1Y, op=AOT.add)
    tt(out=S1, in0=S1, in1=S2, op=AOT.mult)
    ts(MSK, S1, 0.999, None, AOT.is_gt)

    # slot-x weights at xs = clip(x0, 0, 254):
    #   wsx0 = p0x*[x0==xs] + p1x*[x0==-1]
    #   wsx1 = p1x*[x0==xs] + p0x*[x0==255]
    ts(S1, X0, 0.0, 254.0, AOT.max, AOT.min)
    tt(out=S1, in0=X0, in1=S1, op=AOT.is_equal)              # e0x
    tt(out=S2, in0=P0X, in1=S1, op=AOT.mult)
    ts(S3, X0, -1.0, None, AOT.is_equal)
    tt(out=S3, in0=P1X, in1=S3, op=AOT.mult)
    tt(out=WSA, in0=S2, in1=S3, op=AOT.add)
    tt(out=S2, in0=P1X, in1=S1, op=AOT.mult)
    ts(S3, X0, 255.0, None, AOT.is_equal)
    tt(out=S3, in0=P0X, in1=S3, op=AOT.mult)
    tt(out=WSB, in0=S2, in1=S3, op=AOT.add)

    # slot-y weights (mask folded) at ys = clip(y0, 0, 126):
    #   wsy0 = (p0y*[y0==ys] + p1y*[y0==-1]) * m
    #   wsy1 = (p1y*[y0==ys] + p0y*[y0==127]) * m
    ts(S1, Y0, 0.0, 126.0, AOT.max, AOT.min)
    tt(out=S1, in0=Y0, in1=S1, op=AOT.is_equal)              # e0y
    tt(out=S2, in0=P0Y, in1=S1, op=AOT.mult)
    ts(S3, Y0, -1.0, None, AOT.is_equal)
    tt(out=S3, in0=P1Y, in1=S3, op=AOT.mult)
    tt(out=S2, in0=S2, in1=S3, op=AOT.add)                   # wsy0 raw
    tt(out=S1, in0=P1Y, in1=S1, op=AOT.mult)                 # p1y*e0y
    ts(S3, Y0, 127.0, None, AOT.is_equal)
    tt(out=S3, in0=P0Y, in1=S3, op=AOT.mult)                 # p0y*e127
    tt(out=S1, in0=S1, in1=S3, op=AOT.add)                   # wsy1 raw
    tt(out=P0Y, in0=S2, in1=MSK, op=AOT.mult)                # wsy0*m
    tt(out=P1Y, in0=S1, in1=MSK, op=AOT.mult)                # wsy1*m

    # pair weights: W4[i, 0:512] = (y0x0, y0x1) interleaved,
    #               W4[i, 512:1024] = (y1x0, y1x1) interleaved
    w4a = W4[:, 0:512].rearrange("p (n k) -> p n k", k=2)
    w4b = W4[:, 512:1024].rearrange("p (n k) -> p n k", k=2)
    tt(out=w4a[:, :, 0], in0=P0Y, in1=WSA, op=AOT.mult)
    tt(out=w4a[:, :, 1], in0=P0Y, in1=WSB, op=AOT.mult)
    tt(out=w4b[:, :, 0], in0=P1Y, in1=WSA, op=AOT.mult)
    tt(out=w4b[:, :, 1], in0=P1Y, in1=WSB, op=AOT.mult)

    # ---- wrapped-layout flat index map (global, f32) ----
    FXW, FYW, AW, GA = wb[0], wb[1], wb[2], wb[3]
    srcx = flow[0].rearrange("i (q p) -> p (i q)", p=16)
    srcy = flow[1].rearrange("i (q p) -> p (i q)", p=16)
    for g in range(8):
        nc.sync.dma_start(out=FXW[16 * g:16 * (g + 1), :], in_=srcx)
        nc.sync.dma_start(out=FYW[16 * g:16 * (g + 1), :], in_=srcy)
    tt(out=FXW, in0=FXW, in1=cs["JW"][:, :], op=AOT.add)
    tt(out=FYW, in0=FYW, in1=cs["IW"][:, :], op=AOT.add)
    ts(AW, FXW, 8388608.0, 8388608.0, AOT.add, AOT.subtract)
    tt(out=GA, in0=AW, in1=FXW, op=AOT.is_gt)
    tt(out=FXW, in0=AW, in1=GA, op=AOT.subtract)             # x0
    ts(AW, FYW, 8388608.0, 8388608.0, AOT.add, AOT.subtract)
    tt(out=GA, in0=AW, in1=FYW, op=AOT.is_gt)
    tt(out=FYW, in0=AW, in1=GA, op=AOT.subtract)             # y0
    ts(FXW, FXW, 0.0, 254.0, AOT.max, AOT.min)               # xs
    ts(FYW, FYW, 0.0, 126.0, AOT.max, AOT.min)               # ys
    ts(GA, FYW, 256.0, None, AOT.mult)
    tt(out=GA, in0=GA, in1=FXW, op=AOT.add)                  # ys*256 + xs

    # ---------------- gather + blend ----------------
    for (o0, o1, w0) in PASSES:
        nro = o1 - o0
        ncols = nro * 16
        # pass-local uint16 index maps (iB = iA + 256)
        ts(AW[:, 0:ncols], GA[:, o0 * 16:o1 * 16], float(-256 * w0),
           None, AOT.add)
        nc.vector.tensor_copy(out=iA[:, 0:ncols], in_=AW[:, 0:ncols])
        ts(AW[:, 0:ncols], AW[:, 0:ncols], 256.0, None, AOT.add)
        nc.vector.tensor_copy(out=iB[:, 0:ncols], in_=AW[:, 0:ncols])

        for (ch0, nch) in CHUNKS:
            itile = ipool.tile([128, WIN_MAX * 256], F32, tag="img", name="img")
            if nch < 128:
                # quadrant-aligned memset of the garbage partitions; the
                # load below overwrites the valid overlap
                nc.gpsimd.memset(itile[64:128, :], 0.0)
            nc.sync.dma_start(
                out=itile[0:nch, :],
                in_=img[ch0:ch0 + nch,
                        w0 * 256:(w0 + WIN_MAX) * 256])
            dview = itile[:, :].rearrange("p (n k) -> p n k", k=2)

            for h in range(nro // CALL_ROWS):
                r0 = o0 + h * CALL_ROWS          # global first output row
                rl = r0 - o0                     # pass-local first row
                nidx = CALL_ROWS * 256
                half = CALL_ROWS * 512
                G4 = gpool.tile([128, CALL_ROWS * 1024], F32, tag="G4", name="G4")
                # planar pair halves: [0:half] = (y0,x0/x1) pairs,
                # [half:2*half] = (y1,x0/x1) pairs, both in raster order.
                # ISA limit: <=1024 dst elements per IndirectCopy -> 2 rows
                # (512 indices x 2 elems) per call.
                for rr in range(0, CALL_ROWS, 2) if "ic" not in skip else ():
                    c0 = (rl + rr) * 16
                    nc.gpsimd.indirect_copy(
                        out=G4[:, rr * 512:(rr + 2) * 512].rearrange(
                            "p (n k) -> p n k", k=2),
                        data=dview,
                        idxs=iA[:, c0:c0 + 32],
                        i_know_ap_gather_is_preferred=True)
                    nc.gpsimd.indirect_copy(
                        out=G4[:, half + rr * 512:half + (rr + 2) * 512
                               ].rearrange("p (n k) -> p n k", k=2),
                        data=dview,
                        idxs=iB[:, c0:c0 + 32],
                        i_know_ap_gather_is_preferred=True)

                O = opool.tile([128, CALL_ROWS * 256], F32, tag="O", name="O")
                if "blend" in skip:
                    nc.vector.tensor_scalar(O[:, :], G4[:, 0:CALL_ROWS * 256],
                                            1.0, None, AOT.mult)
                for r in range(CALL_ROWS) if "blend" not in skip else ():
                    i_glob = r0 + r
                    rowA = G4[:, r * 512:(r + 1) * 512]
                    rowB = G4[:, half + r * 512:half + (r + 1) * 512]
                    if "pe" not in skip:
                        wstage = wpool.tile([1, 1024], F32, tag="wstage", name="wstage")
                        nc.sync.dma_start(out=wstage[0:1, :],
                                          in_=W4[i_glob:i_glob + 1, :])
                        psum = pspool.tile([128, 1024], F32, tag="wpsum", name="wpsum")
                        nc.tensor.matmul(out=psum[:, 0:512],
                                         lhsT=cs["ONES"][:, :],
                                         rhs=wstage[0:1, 0:512],
                                         start=True, stop=True)
                        nc.tensor.matmul(out=psum[:, 512:1024],
                                         lhsT=cs["ONES"][:, :],
                                         rhs=wstage[0:1, 512:1024],
                                         start=True, stop=True)
                        tt(out=rowA, in0=rowA, in1=psum[:, 0:512], op=AOT.mult)
                        tt(out=rowB, in0=rowB, in1=psum[:, 512:1024], op=AOT.mult)
                    else:
                        tt(out=rowA, in0=rowA, in1=rowA, op=AOT.mult)
                        tt(out=rowB, in0=rowB, in1=rowB, op=AOT.mult)
                    # pair sums in place over each half's front (writes
                    # trail reads on DVE), then cross-half add
                    pA = rowA.rearrange("p (n k) -> p n k", k=2)
                    tt(out=G4[:, r * 512:r * 512 + 256],
                       in0=pA[:, :, 0], in1=pA[:, :, 1], op=AOT.add)
                    pB = rowB.rearrange("p (n k) -> p n k", k=2)
                    tt(out=G4[:, half + r * 512:half + r * 512 + 256],
                       in0=pB[:, :, 0], in1=pB[:, :, 1], op=AOT.add)
                    tt(out=O[:, r * 256:(r + 1) * 256],
                       in0=G4[:, r * 512:r * 512 + 256],
                       in1=G4[:, half + r * 512:half + r * 512 + 256],
                       op=AOT.add)

                nc.sync.dma_start(
                    out=out[ch0:ch0 + nch,
                            r0 * 256:r0 * 256 + nidx],
                    in_=O[0:nch, :])


# ---------------- host side ----------------

_CONSTS = None
_PROGRAM = None


def _host_consts():
    global _CONSTS
    if _CONSTS is None:
        j = np.broadcast_to(np.arange(W, dtype=np.float32), (128, W)).copy()
        i = np.broadcast_to(np.arange(H, dtype=np.float32)[:, None],
                            (H, W)).copy()
        # wrapped layout: [16g+p, i*16+q] = value at pixel (i, q*16+p)
        jw = np.zeros((128, 2048), np.float32)
        iw = np.zeros((128, 2048), np.float32)
        for g in range(8):
            for p in range(16):
                cols = (np.arange(16) * 16 + p).astype(np.float32)
                jw[16 * g + p] = np.tile(cols, H)
                iw[16 * g + p] = np.repeat(np.arange(H), 16).astype(np.float32)
        ones = np.ones((1, 128), np.float32)
        _CONSTS = {"cJ": j, "cI": i, "cJW": jw, "cIW": iw, "cONES": ones}
    return _CONSTS


def make_in_maps(tensorInput, tensorFlow):
    consts = _host_consts()
    in_maps = []
    for b in range(B):
        m = {"img": np.ascontiguousarray(tensorInput[b].reshape(C, HW)),
             "flow": np.ascontiguousarray(tensorFlow[b])}
        m.update(consts)
        in_maps.append(m)
    return in_maps


def kernel(tensorInput, tensorFlow):
    from concourse.bass_utils import run_bass_kernel_spmd

    tensorInput = np.asarray(tensorInput, dtype=np.float32)
    tensorFlow = np.asarray(tensorFlow, dtype=np.float32)
    assert tensorInput.shape == (B, C, H, W)
    assert tensorFlow.shape == (B, 2, H, W)
    # row-window margin envelope (always true for N(0,1) flow)
    if np.abs(tensorFlow[:, 1]).max() >= MARGIN - 0.001:
        return _numpy_reference(tensorInput, tensorFlow)

    global _PROGRAM
    if _PROGRAM is None:
        _PROGRAM = build_program(n_iters=1)
    nc = _PROGRAM

    res = run_bass_kernel_spmd(nc, make_in_maps(tensorInput, tensorFlow),
                               list(range(N_CORES)))
    return np.stack([res.results[b]["out"].reshape(C, H, W)
                     for b in range(B)], axis=0)


def _numpy_reference(tensorInput, tensorFlow):
    """Safety net for out-of-envelope flow magnitudes (not hit by the
    benchmark inputs)."""
    b, c, h, w = tensorInput.shape
    ones = np.ones((b, 1, h, w), np.float32)
    inp = np.concatenate([tensorInput, ones], axis=1)
    gx = np.arange(w, dtype=np.float32)[None, None, :] + tensorFlow[:, 0]
    gy = np.arange(h, dtype=np.float32)[None, :, None] + tensorFlow[:, 1]
    x0 = np.floor(gx)
    y0 = np.floor(gy)
    wx1 = gx - x0
    wy1 = gy - y0
    outv = np.zeros((b, c + 1, h, w), np.float32)
    bidx = np.arange(b)[:, None, None]
    for dy, wy in ((0, 1.0 - wy1), (1, wy1)):
        for dx, wx in ((0, 1.0 - wx1), (1, wx1)):
            xi = x0 + dx
            yi = y0 + dy
            valid = (xi >= 0) & (xi <= w - 1) & (yi >= 0) & (yi <= h - 1)
            xc = np.clip(xi, 0, w - 1).astype(np.int64)
            yc = np.clip(yi, 0, h - 1).astype(np.int64)
            v = inp[bidx, :, yc, xc]            # [b, h, w, c+1]
            v = v * valid[..., None]
            outv += (v * (wx * wy)[..., None]).transpose(0, 3, 1, 2)
    mask = (outv[:, -1:] > 0.999).astype(np.float32)
    return outv[:, :-1] * mask



# revision 15
# speedup vs baseline: 754.1380x; 754.1380x over previous
"""Trainium2 Bass kernel for nn_Backward_14328010900205.

Flow-warp (grid_sample bilinear, zero padding, align_corners=True) with a
hard validity mask, matching the jax reference:

    (y, x) = (i + fy[b,i,j], j + fx[b,i,j])
    out[b,c,i,j] = mask(b,i,j) * sum_k w_k(b,i,j) * img[b,c, y_k, x_k]

Sharding: pure data parallel, one batch sample per NeuronCore (B=8 -> 8 cores).

Device algorithm (per core):
  - Host ships a pixel-major fp16 copy of the sample: imgT[y*W+x, c],
    channels padded 196 -> 256 so each pixel row is 512B.
  - Map phase (DVE, natural [row, col] layout): bilinear corner weights with
    zero-pad masking folded in, and flat gather indices idx = ys*256+xs plus
    corner offsets {0, 1, 256, 257}.
  - PE transposes rearrange the natural-layout maps into (a) the 16-partition
    "wrapped" int16 index layout dma_gather wants and (b) per-block weight
    columns WT[:, blk].
  - dma_gather (SWDGE descriptor gen on GPSIMD, transfer on the 16 DMA
    engines) fetches one 512B pixel row per (pixel, corner) from HBM:
    partition k of gather block blk holds corner k%4 of pixel k//4.
  - Weighting: G *= WT[:, blk] broadcast along free dim (alternating DVE /
    Scalar engine), then one matmul per block with a constant one-hot
    S[k, m] = [k//4 == m] reduces the 4 corners: psum[m, ch].
  - PSUM -> DRAM DMA assembles the pixel-major f32 output outT[px, c];
    host transposes back to [C, H, W].
"""

import numpy as np

from concourse import bacc, bass, mybir, tile

AOT = mybir.AluOpType
F32 = mybir.dt.float32
F16 = mybir.dt.float16
I16 = mybir.dt.int16

B, C, H, W = 8, 196, 128, 256
HW = H * W
CP = 256          # padded channel count (512B fp16 rows)
N_CORES = 8

# gather/block geometry
NU = 8            # transpose blocks: u indexes groups of 32 pixels per row
NRQ = 4           # row-quarters per u  -> 32 gather chunks per sample
ROWS_PER_CHUNK = H // NRQ          # 32 blocks (rows) per chunk
IDX_PER_ROW = 128                  # 32 px * 4 corners
CHUNK_IDX = ROWS_PER_CHUNK * IDX_PER_ROW   # 4096 indices per dma_gather
SCOLS = HW * 4 // 16               # 8192 wrapped idx columns per sample


def split_drain_waits(nc, max_waits=1):
    """walrus CoreV3 codegen rejects instructions carrying more than a
    couple of sync waits; hoist extras onto preceding NoOps."""
    fn = nc.main_func
    n = 0
    for bb in fn.blocks:
        insts = bb.instructions
        i = 0
        while i < len(insts):
            ins = insts[i]
            if type(ins).__name__ != "InstNoOp":
                si = ins.sync_info
                ow = list(si.on_wait) if (si and si.on_wait) else []
                if len(ow) > max_waits:
                    keep = ow[-max_waits:]
                    extras = ow[:-max_waits]
                    ins.sync_info = si.__replace__(on_wait=keep)
                    for k, wt in enumerate(extras):
                        nop = mybir.InstNoOp(
                            name=f"{ins.name}-wsplit{k}",
                            engine=ins.engine,
                            ins=[],
                            outs=[],
                            sync_info=mybir.SyncInfo(on_wait=[wt], on_update=[]),
                        )
                        insts.insert(i, nop)
                        i += 1
                        n += 1
            i += 1
    return n


class Slots:
    """Column-sliced scratch slots inside one SBUF tile."""

    def __init__(self, tilebuf, width):
        self.t = tilebuf
        self.w = width

    def __getitem__(self, k):
        return self.t[:, k * self.w:(k + 1) * self.w]


def build_program(n_iters=1, split_drains=True, skip=()):
    nc = bacc.Bacc("TRN2", target_bir_lowering=False, debug=False,
                   num_swdge_queues=4)

    imgT = nc.declare_dram_parameter("imgT", [HW, CP], F16, isOutput=False)
    flow = nc.declare_dram_parameter("flow", [2, H, W], F32, isOutput=False)
    Jc = nc.declare_dram_parameter("cJ", [128, 256], F32, isOutput=False)
    Ic = nc.declare_dram_parameter("cI", [128, 256], F32, isOutput=False)
    IDc = nc.declare_dram_parameter("cID", [128, 128], F32, isOutput=False)
    Sc = nc.declare_dram_parameter("cS", [128, 32], F16, isOutput=False)
    outT = nc.declare_dram_parameter("outT", [HW, C], F32, isOutput=True)

    with tile.TileContext(nc) as tc:
        with (
            tc.tile_pool(name="consts", bufs=1) as cpool,
            tc.tile_pool(name="maps", bufs=1) as mpool,
            tc.tile_pool(name="gath", bufs=3) as gpool,
            tc.tile_pool(name="outp", bufs=4) as opool,
            tc.tile_pool(name="pst", bufs=2, space="PSUM") as tpool,
            tc.tile_pool(name="psb", bufs=3, space="PSUM") as bpool,
        ):
            cs = {}
            for name, dram, shape, dt in (
                ("J", Jc, [128, 256], F32),
                ("I", Ic, [128, 256], F32),
                ("ID", IDc, [128, 128], F32),
                ("S", Sc, [128, 32], F16),
            ):
                cs[name] = cpool.tile(shape, dt, tag="c" + name, name="c" + name)
                nc.sync.dma_start(out=cs[name][:, :], in_=dram[:, :])

            for _ in range(n_iters):
                _iteration(nc, mpool, gpool, opool, tpool, bpool,
                           imgT, flow, outT, cs, skip)

    nc.compile()  # lowers pseudo-insts, inserts GPSIMD library loads
    if split_drains:
        split_drain_waits(nc)
    return nc


def _iteration(nc, mpool, gpool, opool, tpool, bpool, imgT, flow, outT, cs,
               skip=()):
    ts = nc.vector.tensor_scalar
    tt = nc.vector.tensor_tensor

    # ---------------- map phase (natural [row, col] layout) ----------------
    mb = Slots(mpool.tile([128, 14 * 256], F32, tag="mapbuf", name="mapbuf"), 256)
    Mw = mpool.tile([128, 1024], F32, tag="Mw", name="Mw")
    Mi = mpool.tile([128, 1024], F32, tag="Mi", name="Mi")
    WT = mpool.tile([128, 1024], F16, tag="WT", name="WT")
    IX = mpool.tile([128, SCOLS], I16, tag="IX", name="IX")

    X, Y, AX, AY, X0, Y0 = mb[0], mb[1], mb[2], mb[3], mb[4], mb[5]
    P0X, P1X, P0Y, P1Y, MSK = mb[6], mb[7], mb[8], mb[9], mb[10]
    S1, S2, S3 = mb[11], mb[12], mb[13]
    WSA, WSB = P0X, P1X  # reused after pair weights are folded

    nc.sync.dma_start(out=X, in_=flow[0])
    nc.sync.dma_start(out=Y, in_=flow[1])

    tt(out=X, in0=X, in1=cs["J"][:, :], op=AOT.add)          # x = j + fx
    tt(out=Y, in0=Y, in1=cs["I"][:, :], op=AOT.add)          # y = i + fy
    # floor via round-to-nearest bias trick: r = (x + 2^23) - 2^23;
    # floor = r - [r > x]; frac = x - floor
    ts(X0, X, 8388608.0, 8388608.0, AOT.add, AOT.subtract)   # round(x)
    tt(out=AX, in0=X0, in1=X, op=AOT.is_gt)
    tt(out=X0, in0=X0, in1=AX, op=AOT.subtract)              # floor(x)
    tt(out=AX, in0=X, in1=X0, op=AOT.subtract)               # frac(x)
    ts(Y0, Y, 8388608.0, 8388608.0, AOT.add, AOT.subtract)
    tt(out=AY, in0=Y0, in1=Y, op=AOT.is_gt)
    tt(out=Y0, in0=Y0, in1=AY, op=AOT.subtract)
    tt(out=AY, in0=Y, in1=Y0, op=AOT.subtract)

    # p0x = (1-ax)*[0<=x0<=255], p1x = ax*[-1<=x0<=254]
    ts(S1, X0, 0.0, 255.0, AOT.max, AOT.min)
    tt(out=S1, in0=X0, in1=S1, op=AOT.is_equal)
    ts(S2, AX, -1.0, 1.0, AOT.mult, AOT.add)
    tt(out=P0X, in0=S1, in1=S2, op=AOT.mult)
    ts(S1, X0, -1.0, 254.0, AOT.max, AOT.min)
    tt(out=S1, in0=X0, in1=S1, op=AOT.is_equal)
    tt(out=P1X, in0=AX, in1=S1, op=AOT.mult)
    # p0y, p1y
    ts(S1, Y0, 0.0, 127.0, AOT.max, AOT.min)
    tt(out=S1, in0=Y0, in1=S1, op=AOT.is_equal)
    ts(S2, AY, -1.0, 1.0, AOT.mult, AOT.add)
    tt(out=P0Y, in0=S1, in1=S2, op=AOT.mult)
    ts(S1, Y0, -1.0, 126.0, AOT.max, AOT.min)
    tt(out=S1, in0=Y0, in1=S1, op=AOT.is_equal)
    tt(out=P1Y, in0=AY, in1=S1, op=AOT.mult)

    # mask = [(p0x+p1x)*(p0y+p1y) > 0.999]
    tt(out=S1, in0=P0X, in1=P1X, op=AOT.add)
    tt(out=S2, in0=P0Y, in1=P1Y, op=AOT.add)
    tt(out=S1, in0=S1, in1=S2, op=AOT.mult)
    ts(MSK, S1, 0.999, None, AOT.is_gt)

    # slot-x weights at xs = clip(x0, 0, 254):
    #   wsx0 = p0x*[x0==xs] + p1x*[x0==-1]
    #   wsx1 = p1x*[x0==xs] + p0x*[x0==255]
    ts(S1, X0, 0.0, 254.0, AOT.max, AOT.min)                 # xs (kept in S1? no)
    XS = AX  # frac no longer needed; reuse as xs
    ts(XS, X0, 0.0, 254.0, AOT.max, AOT.min)                 # xs
    tt(out=S1, in0=X0, in1=XS, op=AOT.is_equal)              # e0x
    tt(out=S2, in0=P0X, in1=S1, op=AOT.mult)
    ts(S3, X0, -1.0, None, AOT.is_equal)
    tt(out=S3, in0=P1X, in1=S3, op=AOT.mult)
    tt(out=S2, in0=S2, in1=S3, op=AOT.add)                   # wsx0 raw
    tt(out=S1, in0=P1X, in1=S1, op=AOT.mult)                 # p1x*e0x
    ts(S3, X0, 255.0, None, AOT.is_equal)
    tt(out=S3, in0=P0X, in1=S3, op=AOT.mult)
    tt(out=S1, in0=S1, in1=S3, op=AOT.add)                   # wsx1 raw
    nc.vector.tensor_copy(out=WSA, in_=S2)                   # overwrites P0X
    nc.vector.tensor_copy(out=WSB, in_=S1)                   # overwrites P1X

    # slot-y weights (mask folded) at ys = clip(y0, 0, 126):
    #   wy0 = (p0y*[y0==ys] + p1y*[y0==-1]) * m
    #   wy1 = (p1y*[y0==ys] + p0y*[y0==127]) * m
    YS = AY
    ts(YS, Y0, 0.0, 126.0, AOT.max, AOT.min)                 # ys
    tt(out=S1, in0=Y0, in1=YS, op=AOT.is_equal)              # e0y
    tt(out=S2, in0=P0Y, in1=S1, op=AOT.mult)
    ts(S3, Y0, -1.0, None, AOT.is_equal)
    tt(out=S3, in0=P1Y, in1=S3, op=AOT.mult)
    tt(out=S2, in0=S2, in1=S3, op=AOT.add)                   # wy0 raw
    tt(out=S1, in0=P1Y, in1=S1, op=AOT.mult)                 # p1y*e0y
    ts(S3, Y0, 127.0, None, AOT.is_equal)
    tt(out=S3, in0=P0Y, in1=S3, op=AOT.mult)
    tt(out=S1, in0=S1, in1=S3, op=AOT.add)                   # wy1 raw
    tt(out=P0Y, in0=S2, in1=MSK, op=AOT.mult)                # wy0
    tt(out=P1Y, in0=S1, in1=MSK, op=AOT.mult)                # wy1
    WY0, WY1 = P0Y, P1Y

    # base flat index = ys*256 + xs
    BASE = S2
    ts(BASE, YS, 256.0, None, AOT.mult)
    tt(out=BASE, in0=BASE, in1=XS, op=AOT.add)

    # natural-layout per-corner planes, col = 4*j + z, z in {y0x0,y0x1,y1x0,y1x1}
    Wv = Mw[:, :].rearrange("p (j z) -> p j z", z=4)
    tt(out=Wv[:, :, 0], in0=WY0, in1=WSA, op=AOT.mult)
    tt(out=Wv[:, :, 1], in0=WY0, in1=WSB, op=AOT.mult)
    tt(out=Wv[:, :, 2], in0=WY1, in1=WSA, op=AOT.mult)
    tt(out=Wv[:, :, 3], in0=WY1, in1=WSB, op=AOT.mult)
    Iv = Mi[:, :].rearrange("p (j z) -> p j z", z=4)
    ts(Iv[:, :, 0], BASE, 0.0, None, AOT.add)
    ts(Iv[:, :, 1], BASE, 1.0, None, AOT.add)
    ts(Iv[:, :, 2], BASE, 256.0, None, AOT.add)
    ts(Iv[:, :, 3], BASE, 257.0, None, AOT.add)

    # ------------- transpose maps into gather layouts -------------
    # weights: WT[:, 128u + r] = w at partition delta = 4*(j-32u)+z
    for u in range(NU):
        psw = tpool.tile([128, 128], F32, tag="psw", name="psw")
        nc.tensor.transpose(psw[:, :], Mw[:, 128 * u:128 * (u + 1)],
                            cs["ID"][:, :])
        nc.vector.tensor_copy(out=WT[:, 128 * u:128 * (u + 1)], in_=psw[:, :])
        # indices: for each 16-col group g: psum[p16, r] -> strided int16
        # columns s = 8r + g of the wrapped layout (per-u base 1024u)
        for g in range(8):
            psi = tpool.tile([16, 128], F32, tag="psi", name="psi")
            c0 = 128 * u + 16 * g
            nc.tensor.transpose(psi[:, :], Mi[:, c0:c0 + 16], cs["ID"][:, :])
            dst = IX[0:16, 1024 * u:1024 * (u + 1)].rearrange(
                "p (r g) -> p r g", g=8)
            nc.vector.tensor_copy(out=dst[:, :, g], in_=psi[:, :])
    # replicate the 16-partition wrapped indices to all 8 core groups
    for k in range(1, 8):
        nc.sync.dma_start(out=IX[16 * k:16 * (k + 1), :], in_=IX[0:16, :])

    # ---------------- gather + blend ----------------
    if "gather" in skip:
        return
    # SWDGE caps one gather at ~1024 descriptors (2048 wedges the ring) ->
    # chunk = 1024 indices = 8 blocks (output rows) = one PSUM bank.
    for u in range(NU):
        # weights folded into the stationary one-hot: SW[p, 32*blk + m] =
        # S[p, m] * WT[p, 128u + blk] (free size 32 per block instead of
        # multiplying the 256-wide gathered rows)
        SW = gpool.tile([128, 4096], F16, tag="SW", name="SW")
        if "wmul" not in skip:
            sv = cs["S"][:, :].rearrange("p (a m) -> p a m", a=1)
            wv = WT[:, 128 * u:128 * (u + 1)].rearrange(
                "p (b o) -> p b o", o=1)
            b0, b1 = bass.broadcast_tensor_aps(sv, wv)
            nc.vector.tensor_tensor(
                out=SW[:, :].rearrange("p (b m) -> p b m", m=32),
                in0=b0, in1=b1, op=AOT.mult)
        else:
            nc.vector.memset(SW[:, 0:32], 0.0)
        for R in range(16):
            G = gpool.tile([128, 8, 256], F16, tag="G", name="G")
            scol = 1024 * u + 64 * R
            if "dg" in skip:
                nc.vector.memset(G[:, 0, 0:4], 0.0)  # keep tile alive
            else:
                nc.gpsimd.dma_gather(
                    out_ap=G[:, :, :],
                    in_ap=imgT[:, :],
                    idxs_ap=IX[:, scol:scol + 64],
                    num_idxs=1024,
                    num_idxs_reg=1024,
                    elem_size=CP,
                    queue_num=(16 * u + R) % 4,
                )
            if "blend" in skip:
                continue
            psb = bpool.tile([128, 512], F32, tag="psb", name="psb")
            for q in range(8):
                blk = 8 * R + q
                nc.tensor.matmul(
                    out=psb[32 * (q % 4):32 * (q % 4) + 32,
                            C * (q // 4):C * (q // 4) + C],
                    lhsT=SW[:, 32 * blk:32 * blk + 32],
                    rhs=G[:, q, 0:C],
                    start=True, stop=True,
                    tile_position=(0, 32 * (q % 4)))
            # drain PSUM through SBUF (DMA cannot read PSUM), then
            # 8 blocks -> 8 strided groups of 32 consecutive outT rows;
            # block q = 4*qc + qp sits at partitions 32*qp, col half qc,
            # and covers outT rows 1024*a2 + 256*qp + 32*u + [0, 32).
            ob = opool.tile([128, 2 * C], F32, tag="OB", name="OB")
            if R % 2 == 0:
                nc.scalar.copy(out=ob[:, :], in_=psb[:, 0:2 * C])
            else:
                nc.vector.tensor_copy(out=ob[:, :], in_=psb[:, 0:2 * C])
            for qc in range(2):
                a2 = 2 * R + qc
                dstv = outT[:, :].rearrange(
                    "(a qp u m) c -> a u qp m c", qp=4, u=8, m=32)
                nc.sync.dma_start(out=dstv[a2, u],
                                  in_=ob[:, C * qc:C * qc + C])


# ---------------- host side ----------------

_CONSTS = None
_PROGRAM = None


def _host_consts():
    global _CONSTS
    if _CONSTS is None:
        j = np.broadcast_to(np.arange(W, dtype=np.float32), (128, W)).copy()
        i = np.broadcast_to(np.arange(H, dtype=np.float32)[:, None],
                            (H, W)).copy()
        ident = np.eye(128, dtype=np.float32)
        s = np.zeros((128, 32), np.float16)
        s[np.arange(128), np.arange(128) // 4] = 1.0
        _CONSTS = {"cJ": j, "cI": i, "cID": ident, "cS": s}
    return _CONSTS


def make_in_maps(tensorInput, tensorFlow):
    consts = _host_consts()
    in_maps = []
    for b in range(B):
        imgT = np.zeros((HW, CP), np.float16)
        imgT[:, :C] = tensorInput[b].reshape(C, HW).T
        m = {"imgT": imgT,
             "flow": np.ascontiguousarray(tensorFlow[b])}
        m.update(consts)
        in_maps.append(m)
    return in_maps


def kernel(tensorInput, tensorFlow):
    from concourse.bass_utils import run_bass_kernel_spmd

    tensorInput = np.asarray(tensorInput, dtype=np.float32)
    tensorFlow = np.asarray(tensorFlow, dtype=np.float32)
    assert tensorInput.shape == (B, C, H, W)
    assert tensorFlow.shape == (B, 2, H, W)

    global _PROGRAM
    if _PROGRAM is None:
        _PROGRAM = build_program(n_iters=1)
    nc = _PROGRAM

    res = run_bass_kernel_spmd(nc, make_in_maps(tensorInput, tensorFlow),
                               list(range(N_CORES)))
    return np.stack([np.ascontiguousarray(
        res.results[b]["outT"].T).reshape(C, H, W) for b in range(B)], axis=0)
